# revision 21
# baseline (speedup 1.0000x reference)
"""AttnBlock (GroupNorm -> single-head attention over 64x64 tokens -> proj -> residual)
for Trainium2, SPMD over 8 NeuronCores.

Sharding: core = batch(4) x query-half(2) (token order along j is permutation-
invariant for softmax attention and GroupNorm stats).

All heavy matmuls run in fp8e4m3 with DoubleRow perf mode (contract 256/instr
at 0.5 cycles/row): QKV projections, S^T = k^T q, O = vT e, the softmax
denominator (ones-matmul), and the output projection.

Scaling scheme (all powers of 2, exact):
  weights stored as 8*W^T in fp8; q,k,v carry x8; S_psum = 64*S_true
  exp: et = exp(S_psum * 1/(64*sqrt(C)) - ln16) = e_true/16  (fp8 range safe)
  l_psum = sum(et)/8 = l_true/128 ; lrb = recip = 128/l_true
  o_bf = opsum * lrb = 64*O_norm (fp8) ; proj psum = 512*(Wp O_norm)
  out = ps*(1/512) + (x + bp + Wp bv)

Bias folds: bk dropped exactly (softmax shift invariance); bv folded into
bp_eff = bp + Wp@bv host-side; bq added on the q PSUM->SBUF copy.

Softmax exp is staged: S psum tiles are copied (Pool/DVE) to a bf16 SBUF
buffer of 8 j-chunks, then ONE 4096-wide ACT exp produces fp8 et directly.

Layouts (SBUF, partition dim first):
  h8,k8: [128, 4cc, 4096] channel on partitions, tokens free (fp8)
  q8   : [128, 4cc, 2048]
  vt8  : [128jc, 32, 512] token chunk on partitions, channel free (fp8)
  S^T  : psum [128 j, 512 i]; et: [128 j, 8jc, 512 i] fp8
  O    : psum [128 c, 512 i] accumulated over 16 j-pairs via DoubleRow
"""

import math
import numpy as np
import ml_dtypes

import concourse.bass as bass
import concourse.mybir as mybir
import concourse.tile as tile
from concourse import library_config

P = 128
C = 512
NCC = C // P          # 4 channel chunks
HW = 4096             # tokens per batch image
IHALF = 2048          # query tokens per core
NBLK = IHALF // 512   # 4 i-blocks of 512
NJC = HW // P         # 32 j chunks of 128
NJT = HW // 512       # 8 j tiles of 512
GS = 16               # channels per group
EPS = 1e-6
WS = 8.0
EXP_SCALE = 1.0 / (64.0 * math.sqrt(C))
EXP_BIAS = -math.log(16.0)

F32 = mybir.dt.float32
BF16 = mybir.dt.bfloat16
F8 = mybir.dt.float8e4
BF = ml_dtypes.bfloat16
E4 = ml_dtypes.float8_e4m3

DR = mybir.MatmulPerfMode.DoubleRow
ALU = mybir.AluOpType
AF = mybir.ActivationFunctionType


def _split_excess_waits(nc):
    """walrus in this container accepts only ONE sync-wait per instruction;
    move extra waits onto same-engine NOPs placed immediately before."""
    for fn in nc.m.functions:
        for bb in fn.blocks:
            insts = list(bb.instructions)
            out = []
            changed = False
            for inst in insts:
                si = inst.sync_info
                if si is not None and len(si.on_wait) > 1:
                    waits = list(si.on_wait)
                    for k, w in enumerate(waits[:-1]):
                        nop = mybir.InstNoOp(
                            name=f"{inst.name}-ws{k}",
                            sync_info=mybir.SyncInfo(on_wait=[w], on_update=[]),
                            bass_nofuse=True,
                            engine=inst.engine,
                        )
                        out.append(nop)
                    inst.sync_info = mybir.SyncInfo(
                        on_wait=[waits[-1]], on_update=list(si.on_update)
                    )
                    changed = True
                out.append(inst)
            if changed:
                bb.instructions = out


def build_nc(split_waits=True):
    nc = bass.Bass()

    xbf_d = nc.declare_dram_parameter("x_bf", [C, HW], BF16, isOutput=False)
    xres_d = nc.declare_dram_parameter("x_res", [C, IHALF], F32, isOutput=False)
    wq8_d = nc.declare_dram_parameter("wq8", [C, C], F8, isOutput=False)
    wk8_d = nc.declare_dram_parameter("wk8", [C, C], F8, isOutput=False)
    wv8_d = nc.declare_dram_parameter("wv8", [C, C], F8, isOutput=False)
    wp8_d = nc.declare_dram_parameter("wp8", [C, C], F8, isOutput=False)
    bq8_d = nc.declare_dram_parameter("bq8_pc", [P, NCC], F32, isOutput=False)
    bpe_d = nc.declare_dram_parameter("bpe_pc", [P, NCC], F32, isOutput=False)
    gamma_d = nc.declare_dram_parameter("gamma_pc", [P, NCC], F32, isOutput=False)
    beta_d = nc.declare_dram_parameter("beta_pc", [P, NCC], F32, isOutput=False)
    indh_d = nc.declare_dram_parameter("indh", [P, P // GS], F32, isOutput=False)
    indt_d = nc.declare_dram_parameter("indt", [P, P // GS], F32, isOutput=False)
    bcast16_d = nc.declare_dram_parameter("bcast16", [P // GS, P], F32, isOutput=False)
    y_d = nc.declare_dram_parameter("yout", [C, IHALF], F32, isOutput=True)

    from contextlib import ExitStack

    with tile.TileContext(nc) as tc:
        with ExitStack() as stack:
            wpool = stack.enter_context(tc.tile_pool(name="w", bufs=1))
            cpool = stack.enter_context(tc.tile_pool(name="const", bufs=1))
            hpool = stack.enter_context(tc.tile_pool(name="hbuf", bufs=1))
            kpool = stack.enter_context(tc.tile_pool(name="kbuf", bufs=1))
            vpool = stack.enter_context(tc.tile_pool(name="vbuf", bufs=1))
            qpool = stack.enter_context(tc.tile_pool(name="qbuf", bufs=1))
            wq8 = wpool.tile([P, NCC, C], F8, tag="wq8")
            wk8 = wpool.tile([P, NCC, C], F8, tag="wk8")
            wv8 = wpool.tile([P, NCC, C], F8, tag="wv8")
            wp8 = wpool.tile([P, NCC, C], F8, tag="wp8")

            bq8_sb = cpool.tile([P, NCC], F32, tag="bq8")
            bpe_sb = cpool.tile([P, NCC], F32, tag="bpe")
            gamma_sb = cpool.tile([P, NCC], F32, tag="gamma")
            beta_sb = cpool.tile([P, NCC], F32, tag="beta")
            indh_sb = cpool.tile([P, P // GS], F32, tag="indh")
            indt_sb = cpool.tile([P, P // GS], F32, tag="indt")
            bcast16_sb = cpool.tile([P // GS, P], F32, tag="bcast16")
            eps_sb = cpool.tile([P // GS, 1], F32, tag="eps")
            ebias_sb = cpool.tile([P, 1], F32, tag="ebias")
            ones8_sb = cpool.tile([P, 2, 1], F8, tag="ones8")

            h8 = hpool.tile([P, NCC, HW], F8, tag="h8")
            k8 = kpool.tile([P, NCC, HW], F8, tag="k8")
            vt8 = vpool.tile([P, NJC, C], F8, tag="vt8")
            q8 = qpool.tile([P, NCC, IHALF], F8, tag="q8")

            nc.vector.memset(eps_sb[:], EPS)
            nc.vector.memset(ebias_sb[:], EXP_BIAS)
            nc.vector.memset(ones8_sb[:], 1.0)

            # ====== phase 0: DMA in, GN stats on 3 engines, h8 = fp8(x*sc+sh) ======
            with ExitStack() as stack0:
                xpool = stack0.enter_context(tc.tile_pool(name="xbuf", bufs=1))
                gpool = stack0.enter_context(tc.tile_pool(name="gn", bufs=2))
                gppool = stack0.enter_context(tc.tile_pool(name="gnp", bufs=2, space="PSUM"))
                xb = xpool.tile([P, NCC, HW], BF16, tag="xb")
                half = HW // 2
                # one chunk per DMA queue; two halves each so stats can start early
                for ci, eng in ((0, nc.sync), (1, nc.gpsimd), (2, nc.scalar)):
                    eng.dma_start(out=xb[:, ci, :half], in_=xbf_d[ci * P:(ci + 1) * P, :half])
                    eng.dma_start(out=xb[:, ci, half:], in_=xbf_d[ci * P:(ci + 1) * P, half:])
                nc.sync.dma_start(out=xb[:, 3, :half], in_=xbf_d[3 * P:4 * P, :half])
                nc.scalar.dma_start(out=xb[:, 3, half:], in_=xbf_d[3 * P:4 * P, half:])
                # weights on sync queue (k first), consts on gpsimd queue
                nc.sync.dma_start(out=wk8[:], in_=wk8_d[:].rearrange("(cc p) o -> p cc o", p=P))
                nc.sync.dma_start(out=wq8[:], in_=wq8_d[:].rearrange("(cc p) o -> p cc o", p=P))
                nc.sync.dma_start(out=wv8[:], in_=wv8_d[:].rearrange("(cc p) o -> p cc o", p=P))
                nc.sync.dma_start(out=wp8[:], in_=wp8_d[:].rearrange("(cc p) o -> p cc o", p=P))
                for t, d in (
                    (indh_sb, indh_d), (indt_sb, indt_d), (gamma_sb, gamma_d),
                    (beta_sb, beta_d), (bq8_sb, bq8_d), (bpe_sb, bpe_d),
                    (bcast16_sb, bcast16_d),
                ):
                    nc.gpsimd.dma_start(out=t[:], in_=d[:])

                scale_sb = gpool.tile([P, NCC], F32, tag="scale")
                shift_sb = gpool.tile([P, NCC], F32, tag="shift")
                gpsum = gppool.tile([P // GS, 2 * NCC], F32, tag="gstat")

                for ci in range(NCC):
                    # DVE: bn_stats over tokens 0..2559 (5 blocks of 512)
                    stats = gpool.tile([P, 5, 6], F32, tag="stats")
                    for sg in range(5):
                        nc.vector.bn_stats(
                            out=stats[:, sg, :],
                            in_=xb[:, ci, sg * 512:(sg + 1) * 512],
                        )
                    mv = gpool.tile([P, 2], F32, tag="mv")
                    nc.vector.bn_aggr(out=mv[:], in_=stats[:])
                    # u = [mean, E[x^2]] over the DVE part
                    u = gpool.tile([P, 2], F32, tag="u")
                    nc.vector.tensor_copy(out=u[:, 0:1], in_=mv[:, 0:1])
                    nc.vector.tensor_tensor(u[:, 1:2], mv[:, 0:1], mv[:, 0:1], ALU.mult)
                    nc.vector.tensor_add(u[:, 1:2], u[:, 1:2], mv[:, 1:2])
                    # Pool: raw sums over tokens 2560..4095
                    s_pool = gpool.tile([P, 2], F32, tag="spool")
                    scr2 = gpool.tile([P, 1536], BF16, tag="scr2")
                    nc.gpsimd.scalar_tensor_tensor(
                        out=scr2[:], in0=xb[:, ci, 2560:4096], scalar=1.0,
                        in1=xb[:, ci, 2560:4096], op0=ALU.mult, op1=ALU.bypass,
                        accum_out=s_pool[:, 0:1],
                    )
                    nc.gpsimd.scalar_tensor_tensor(
                        out=scr2[:], in0=xb[:, ci, 2560:4096], scalar=1.0,
                        in1=xb[:, ci, 2560:4096], op0=ALU.mult, op1=ALU.mult,
                        accum_out=s_pool[:, 1:2],
                    )
                    # group-reduce: indh has (5/8)/16 (DVE means), indt 1/(16*4096)
                    gsl = gpsum[:, ci * 2:(ci + 1) * 2]
                    nc.tensor.matmul(gsl, lhsT=indh_sb[:], rhs=u[:], start=True, stop=False)
                    nc.tensor.matmul(gsl, lhsT=indt_sb[:], rhs=s_pool[:], start=False, stop=True)

                    # group mean/rstd -> broadcast -> per-channel scale/shift
                    gmr = gpool.tile([P // GS, 2], F32, tag="gmr", name=f"gmr{ci}")
                    nc.vector.tensor_copy(out=gmr[:], in_=gsl)
                    mu = gmr[:, 0:1]
                    var = gmr[:, 1:2]
                    tmpv = gpool.tile([P // GS, 1], F32, tag="tmpv")
                    nc.vector.tensor_tensor(tmpv[:], mu, mu, ALU.mult)
                    nc.vector.tensor_tensor(var, var, tmpv[:], ALU.subtract)
                    nc.scalar.activation(
                        out=var, in_=var, func=AF.Sqrt, bias=eps_sb[:], scale=1.0
                    )
                    nc.vector.reciprocal(out=var, in_=var)
                    bpsum = gppool.tile([P, 2], F32, tag="bc")
                    nc.tensor.matmul(
                        bpsum[:], lhsT=bcast16_sb[:], rhs=gmr[:], start=True, stop=True
                    )
                    sc = scale_sb[:, ci:ci + 1]
                    sh = shift_sb[:, ci:ci + 1]
                    nc.vector.tensor_tensor(
                        sc, bpsum[:, 1:2], gamma_sb[:, ci:ci + 1], ALU.mult
                    )
                    nc.vector.tensor_tensor(sh, bpsum[:, 0:1], sc, ALU.mult)
                    nc.vector.tensor_tensor(
                        sh, beta_sb[:, ci:ci + 1], sh, ALU.subtract
                    )
                    # h8 = x*sc + sh in fp8; halves on Pool + (ACT for ci<2 else DVE)
                    nc.gpsimd.tensor_scalar(
                        out=h8[:, ci, :half], in0=xb[:, ci, :half],
                        scalar1=sc, scalar2=sh, op0=ALU.mult, op1=ALU.add,
                    )
                    if ci < 2:
                        nc.scalar.activation(
                            out=h8[:, ci, half:], in_=xb[:, ci, half:],
                            func=AF.Identity, bias=sh, scale=sc,
                        )
                    else:
                        nc.vector.tensor_scalar(
                            out=h8[:, ci, half:], in0=xb[:, ci, half:],
                            scalar1=sc, scalar2=sh, op0=ALU.mult, op1=ALU.add,
                        )


            # ====== phase 1: QKV projections (DoubleRow fp8) ======
            # PSUM->SBUF fp8 conversion copies rotate over Pool/DVE/ACT (ACT is
            # idle during this phase); bias is folded into the copy where needed.
            ncpy = [0]

            def cpy3_engine():
                ncpy[0] += 1
                return (nc.vector, nc.vector, nc.scalar)[ncpy[0] % 3]

            def copy_to(eng, dst, src, bias=None):
                if eng is nc.scalar:
                    if bias is None:
                        eng.activation(out=dst, in_=src, func=AF.Copy)
                    else:
                        eng.activation(out=dst, in_=src, func=AF.Identity, bias=bias)
                elif eng is nc.gpsimd:
                    eng.tensor_scalar(
                        out=dst, in0=src,
                        scalar1=(0.0 if bias is None else bias), scalar2=None,
                        op0=ALU.add,
                    )
                else:
                    if bias is None:
                        eng.tensor_copy(out=dst, in_=src)
                    else:
                        eng.tensor_scalar(
                            out=dst, in0=src, scalar1=bias, scalar2=None,
                            op0=ALU.add,
                        )

            with tc.tile_pool(name="mmp", bufs=4, space="PSUM") as mmpool:

                def emit_k(jtp):
                    for oc in range(NCC):
                        ps = mmpool.tile([P, 2, 512], F32, tag="mm")
                        for t in range(2):
                            jt = jtp * 2 + t
                            for g in range(2):
                                nc.tensor.matmul(
                                    ps[:, t, :],
                                    lhsT=wk8[:, 2 * g:2 * g + 2, oc * P:(oc + 1) * P],
                                    rhs=h8[:, 2 * g:2 * g + 2, jt * 512:(jt + 1) * 512],
                                    start=(g == 0), stop=(g == 1), perf_mode=DR,
                                )
                        copy_to(
                            cpy3_engine(),
                            k8[:, oc, jtp * 1024:(jtp + 1) * 1024], ps[:, :, :],
                        )

                def emit_q(itp):
                    for oc in range(NCC):
                        ps = mmpool.tile([P, 2, 512], F32, tag="mm")
                        for t in range(2):
                            it = itp * 2 + t
                            for g in range(2):
                                nc.tensor.matmul(
                                    ps[:, t, :],
                                    lhsT=wq8[:, 2 * g:2 * g + 2, oc * P:(oc + 1) * P],
                                    rhs=h8[:, 2 * g:2 * g + 2, it * 512:(it + 1) * 512],
                                    start=(g == 0), stop=(g == 1), perf_mode=DR,
                                )
                        copy_to(
                            cpy3_engine(),
                            q8[:, oc, itp * 1024:(itp + 1) * 1024], ps[:, :, :],
                            bias=bq8_sb[:, oc:oc + 1],
                        )

                def emit_v(jcp):
                    ps = mmpool.tile([P, 2, 512], F32, tag="mm")
                    for t in range(2):
                        jc = jcp * 2 + t
                        for g in range(2):
                            nc.tensor.matmul(
                                ps[:, t, :],
                                lhsT=h8[:, 2 * g:2 * g + 2, jc * P:(jc + 1) * P],
                                rhs=wv8[:, 2 * g:2 * g + 2, :],
                                start=(g == 0), stop=(g == 1), perf_mode=DR,
                            )
                    copy_to(cpy3_engine(), vt8[:, jcp * 2:jcp * 2 + 2, :], ps[:, :, :])

                emit_k(0)
                emit_q(0)
                for jcp in range(0, 4):
                    emit_v(jcp)
                emit_k(1)
                for jcp in range(4, 8):
                    emit_v(jcp)
                emit_k(2)
                for jcp in range(8, 12):
                    emit_v(jcp)
                emit_k(3)
                for jcp in range(12, 16):
                    emit_v(jcp)
                emit_q(1)

            # ====== phase 2: attention (+ phase 3 interleaved per i-block) ======
            # S^T is built in [128, 2jc, 512] PSUM pair-tiles; ACT exponentiates
            # straight out of PSUM (1024 wide) into retained fp8 et pair-tiles.
            # O accumulates cc0..2 on the fly (3 banks); cc3 runs as a second
            # pass over the retained et tiles once the l/obf bank frees up.
            with ExitStack() as stack1:
                etpool = stack1.enter_context(tc.tile_pool(name="et", bufs=18))
                obpool = stack1.enter_context(tc.tile_pool(name="ob", bufs=NBLK))
                lbpool = stack1.enter_context(tc.tile_pool(name="lb", bufs=2))
                lrbpool = stack1.enter_context(tc.tile_pool(name="lrb", bufs=2))
                ldpool = stack1.enter_context(tc.tile_pool(name="ld", bufs=2, space="DRAM"))
                stpool = stack1.enter_context(tc.tile_pool(name="stp", bufs=2, space="PSUM"))
                oapool = stack1.enter_context(tc.tile_pool(name="oap", bufs=1, space="PSUM"))
                lpool = stack1.enter_context(tc.tile_pool(name="lp", bufs=1, space="PSUM"))
                xrpool = stack1.enter_context(tc.tile_pool(name="xr", bufs=4))
                ospool = stack1.enter_context(tc.tile_pool(name="os", bufs=4))

                def attn_pair(ib, pair, opsum, lpsum, ets):
                    isl = slice(ib * 512, (ib + 1) * 512)
                    st = stpool.tile([P, 2, 512], F32, tag="st")
                    for t in range(2):
                        jc = pair * 2 + t
                        for gg in range(2):
                            nc.tensor.matmul(
                                st[:, t, :],
                                lhsT=k8[:, 2 * gg:2 * gg + 2, jc * P:(jc + 1) * P],
                                rhs=q8[:, 2 * gg:2 * gg + 2, isl],
                                start=(gg == 0), stop=(gg == 1), perf_mode=DR,
                            )
                    et = etpool.tile([P, 2, 512], F8, tag="et", name=f"et{ib}_{pair}")
                    nc.scalar.activation(
                        out=et[:], in_=st[:], func=AF.Exp,
                        scale=EXP_SCALE, bias=ebias_sb[:],
                    )
                    ets.append(et)
                    first = pair == 0
                    last = pair == 15
                    for cc in range(3):
                        nc.tensor.matmul(
                            opsum[cc][:],
                            lhsT=vt8[:, pair * 2:pair * 2 + 2, cc * P:(cc + 1) * P],
                            rhs=et[:, :, :],
                            start=first, stop=last, perf_mode=DR,
                        )
                    nc.tensor.matmul(
                        lpsum[:],
                        lhsT=ones8_sb[:],
                        rhs=et[:, :, :],
                        start=first, stop=last, perf_mode=DR,
                    )

                def attn_tail(ib, opsum, lpsum, ets):
                    # drain cc0..2 at fixed 1/64 scale, then run the cc3 pass
                    obf = obpool.tile([P, NCC, 512], F8, tag="obf", name=f"obf{ib}")
                    for cc in range(3):
                        nc.vector.tensor_scalar(
                            out=obf[:, cc, :], in0=opsum[cc][:],
                            scalar1=1.0 / 64.0, scalar2=None, op0=ALU.mult,
                        )
                    o3 = oapool.tile([P, 512], F32, tag="o0", name="o3pass")
                    for p, et in enumerate(ets):
                        nc.tensor.matmul(
                            o3[:],
                            lhsT=vt8[:, p * 2:p * 2 + 2, 3 * P:4 * P],
                            rhs=et[:, :, :],
                            start=(p == 0), stop=(p == 15), perf_mode=DR,
                        )
                    nc.vector.tensor_scalar(
                        out=obf[:, 3, :], in0=o3[:],
                        scalar1=1.0 / 64.0, scalar2=None, op0=ALU.mult,
                    )
                    linv = lbpool.tile([1, 512], F32, tag="linv")
                    nc.vector.reciprocal(out=linv[:], in_=lpsum[:])
                    l_dram = ldpool.tile([1, 512], F32, tag="ldram")
                    nc.sync.dma_start(out=l_dram[:], in_=linv[:])
                    lrb = lrbpool.tile([P, 512], F32, tag="lrb", name=f"lrb{ib}")
                    nc.sync.dma_start(out=lrb[:], in_=l_dram[:].to_broadcast((P, 512)))
                    return obf, lrb

                def emit_proj(ib, obf, lrb):
                    # phase 3 for one i-block: out = (Wp @ O)*lrb + (x + bp_eff)
                    isl = slice(ib * 512, (ib + 1) * 512)
                    for oc in range(NCC):
                        xr = xrpool.tile([P, 512], F32, tag="xr")
                        nc.sync.dma_start(
                            out=xr[:], in_=xres_d[oc * P:(oc + 1) * P, isl]
                        )
                        nc.gpsimd.tensor_scalar(
                            out=xr[:], in0=xr[:], scalar1=bpe_sb[:, oc:oc + 1],
                            scalar2=None, op0=ALU.add,
                        )
                        ps = stpool.tile([P, 2, 512], F32, tag="st")
                        for g in range(2):
                            nc.tensor.matmul(
                                ps[:, 0, :],
                                lhsT=wp8[:, 2 * g:2 * g + 2, oc * P:(oc + 1) * P],
                                rhs=obf[:, 2 * g:2 * g + 2, :],
                                start=(g == 0), stop=(g == 1), perf_mode=DR,
                            )
                        tmp = ospool.tile([P, 512], F32, tag="tmp")
                        nc.vector.tensor_tensor(tmp[:], ps[:, 0, :], lrb[:, :], ALU.mult)
                        ost = ospool.tile([P, 512], F32, tag="ost")
                        nc.gpsimd.tensor_tensor(ost[:], tmp[:], xr[:], ALU.add)
                        eng = nc.sync if oc % 2 == 0 else nc.gpsimd
                        eng.dma_start(out=y_d[oc * P:(oc + 1) * P, isl], in_=ost[:])

                o_bfs = []
                for ib in range(NBLK):
                    opsum = [
                        oapool.tile([P, 512], F32, tag=f"o{cc}", name=f"op{cc}")
                        for cc in range(3)
                    ]
                    lpsum = lpool.tile([1, 512], F32, tag="l")
                    ets = []
                    for pair in range(16):
                        attn_pair(ib, pair, opsum, lpsum, ets)
                        if pair == 5 and ib > 0:
                            emit_proj(ib - 1, *o_bfs[ib - 1])
                    o_bfs.append(attn_tail(ib, opsum, lpsum, ets))
                emit_proj(NBLK - 1, *o_bfs[NBLK - 1])

    if split_waits:
        _split_excess_waits(nc)
    return nc


_NC = None


def _get_nc():
    global _NC
    if _NC is None:
        _NC = build_nc()
    return _NC


def _core0_feed(inputs):
    """Input map for core 0 (batch 0, first query half) — used by test harnesses."""
    maps = _build_in_maps(**inputs)
    return maps[0]


def _build_in_maps(x, gamma, beta, Wq, bq, Wk, bk, Wv, bv, Wp, bp):
    x = np.asarray(x, dtype=np.float32)
    B, c, H, W = x.shape
    assert (B, c, H, W) == (4, C, 64, 64)

    def pc(v):  # [C] -> [P, NCC]
        return np.ascontiguousarray(np.asarray(v, np.float32).reshape(NCC, P).T)

    indh = np.zeros((P, P // GS), np.float32)
    indh[np.arange(P), np.arange(P) // GS] = 5.0 / (GS * 8.0)
    indt = np.zeros((P, P // GS), np.float32)
    indt[np.arange(P), np.arange(P) // GS] = 1.0 / (GS * HW)
    bcast16 = np.zeros((P // GS, P), np.float32)
    bcast16[np.arange(P) // GS, np.arange(P)] = 1.0

    Wp32 = np.asarray(Wp, np.float32)
    bv32 = np.asarray(bv, np.float32)
    bp_eff = np.asarray(bp, np.float32) + Wp32 @ bv32

    def w8(wmat):
        return np.ascontiguousarray(
            np.asarray(wmat, np.float32).T * WS
        ).astype(E4)

    shared = {
        "wq8": w8(Wq), "wk8": w8(Wk), "wv8": w8(Wv), "wp8": w8(Wp),
        "bq8_pc": pc(np.asarray(bq, np.float32) * WS),
        "bpe_pc": pc(bp_eff),
        "gamma_pc": pc(gamma), "beta_pc": pc(beta),
        "indh": indh, "indt": indt, "bcast16": bcast16,
    }

    xf = x.reshape(B, C, HW)
    in_maps = []
    for core in range(8):
        b, halfsel = divmod(core, 2)
        xb = xf[b]
        if halfsel == 0:
            x_bc = xb
        else:
            x_bc = np.concatenate([xb[:, IHALF:], xb[:, :IHALF]], axis=1)
        x_bc = np.ascontiguousarray(x_bc)
        in_maps.append({
            "x_bf": x_bc.astype(BF),
            "x_res": np.ascontiguousarray(x_bc[:, :IHALF]),
            **shared,
        })
    return in_maps


def kernel(x, gamma, beta, Wq, bq, Wk, bk, Wv, bv, Wp, bp):
    nc = _get_nc()
    in_maps = _build_in_maps(x, gamma, beta, Wq, bq, Wk, bk, Wv, bv, Wp, bp)

    from concourse.bass_utils import run_bass_kernel_spmd

    res = run_bass_kernel_spmd(nc, in_maps, list(range(8)))

    B = 4
    out = np.empty((B, C, HW), np.float32)
    for core in range(8):
        b, halfsel = divmod(core, 2)
        out[b, :, halfsel * IHALF:(halfsel + 1) * IHALF] = res.results[core]["yout"]
    return out.reshape(B, C, 64, 64)
